# revision 1
# baseline (speedup 1.0000x reference)
"""Decoder layer (attn + FFN + 2 layernorms) on 8 Trainium2 cores.

Sharding: core c handles batch b = c//4, query chunk i = c%4 (512 tokens).
Each core redundantly computes K/V for the full sequence (communication-free).
Causality is handled by rotating the key/value token order per core on the
host (self chunk first, then past, then future) so the mask structure is
uniform across cores: k-tiles 0-3 (the self chunk) get compile-time
triangular masks, the rest get a per-core additive bias (0 for past,
-1e30 for future) folded into the softmax exp. Softmax runs unnormalized
(no max subtraction; scores are O(+-8)) with the denominator taken from an
appended ones-column on V, and the division folded into the context copy.

K/V are computed and consumed chunk-by-chunk (fused with attention) so they
never need full SBUF residency; per-head context accumulates in SBUF across
chunks. The FFN intermediate (d_ff=4096) bounces through DRAM. Score
matmuls (contraction = head_dim 64) run as head PAIRS on disjoint PE
row-strips via tile_position for ~2x concurrency.

All matmuls run in float32r (TF32-like fast fp32 mode: 1 cycle/row at
free-dim >= 256 vs 4 cycles/row for exact fp32).
"""

import sys

sys.path.insert(0, "/opt/trn_rl_repo")

import numpy as np

D = 1024          # d_model
H = 16            # heads
HD = 64           # head dim
DFF = 4096
EPS = 1e-6
B, S = 2, 2048
QCH = 512         # query tokens per core
NCORES = 8
P = 128
NCH = S // QCH            # 4 chunks of k/v tokens
KT_TILES = S // P         # 16 k tiles
NDT = D // P              # 8 d_model tiles
NFT = DFF // P            # 32 d_ff tiles
QT_T = QCH // P           # 4 query token tiles
NEG = -1.0e30

_CACHE = {}


def _build(mm_dtype_name="float32r", debug=False):
    import concourse.bacc as bacc
    import concourse.mybir as mybir
    import concourse.tile as tile
    from concourse.masks import make_identity

    dt = mybir.dt
    MMDT = getattr(dt, mm_dtype_name)
    AF = mybir.ActivationFunctionType
    OP = mybir.AluOpType

    nc = bacc.Bacc("TRN2", target_bir_lowering=False, debug=False)

    # ---- I/O ----
    xb = nc.dram_tensor("xb", [S, D], dt.float32, kind="ExternalInput")
    kbias = nc.dram_tensor("kbias", [P, KT_TILES], dt.float32, kind="ExternalInput")
    Wq = nc.dram_tensor("Wq", [D, D], dt.float32, kind="ExternalInput")
    Wk = nc.dram_tensor("Wk", [D, D], dt.float32, kind="ExternalInput")
    Wv = nc.dram_tensor("Wv", [D, D], dt.float32, kind="ExternalInput")
    Wo = nc.dram_tensor("Wo", [D, D], dt.float32, kind="ExternalInput")
    W1 = nc.dram_tensor("W1", [D, DFF], dt.float32, kind="ExternalInput")
    W2 = nc.dram_tensor("W2", [DFF, D], dt.float32, kind="ExternalInput")
    bq = nc.dram_tensor("bq", [D], dt.float32, kind="ExternalInput")
    bk = nc.dram_tensor("bk", [D], dt.float32, kind="ExternalInput")
    bv = nc.dram_tensor("bv", [D], dt.float32, kind="ExternalInput")
    bo = nc.dram_tensor("bo", [D], dt.float32, kind="ExternalInput")
    b1 = nc.dram_tensor("b1", [DFF], dt.float32, kind="ExternalInput")
    b2 = nc.dram_tensor("b2", [D], dt.float32, kind="ExternalInput")
    g1 = nc.dram_tensor("g1", [D], dt.float32, kind="ExternalInput")
    be1 = nc.dram_tensor("be1", [D], dt.float32, kind="ExternalInput")
    g2 = nc.dram_tensor("g2", [D], dt.float32, kind="ExternalInput")
    be2 = nc.dram_tensor("be2", [D], dt.float32, kind="ExternalInput")
    out = nc.dram_tensor("out", [QCH, D], dt.float32, kind="ExternalOutput")
    dbg = {}
    if debug:
        for nm, shp in [("dbg_xqT", [P, NDT, QCH]), ("dbg_QT", [P, NDT, QCH]),
                        ("dbg_kt", [P, NDT, QCH]), ("dbg_v", [P, QT_T, H, HD + 1]),
                        ("dbg_ctx", [P, NDT, QCH]), ("dbg_cs", [P, 4, QCH]),
                        ("dbg_ctxT", [P, NDT, QCH]), ("dbg_yT", [P, NDT, QCH]),
                        ("dbg_hT", [P, NDT, QCH]), ("dbg_ex", [P, QCH])]:
            dbg[nm] = nc.dram_tensor(nm, shp, dt.float32, kind="ExternalOutput")

    xb3 = xb.rearrange("(c t p) d -> c t p d", t=QT_T, p=P)  # chunk, toktile, p, d
    WqT = Wq.rearrange("(ko p) d -> p ko d", p=P)  # d_in on partitions
    WkT = Wk.rearrange("(ko p) d -> p ko d", p=P)
    WvT = Wv.rearrange("(ko p) d -> p ko d", p=P)
    WoT = Wo.rearrange("(ko p) d -> p ko d", p=P)
    W1T = W1.rearrange("(ko p) f -> p ko f", p=P)
    W2T = W2.rearrange("(ko p) d -> p ko d", p=P)

    with tile.TileContext(nc) as tc:
        with (
            tc.tile_pool(name="consts", bufs=1) as consts,
            tc.tile_pool(name="mid", bufs=4) as mid,
            tc.tile_pool(name="ktb", bufs=1) as ktb_pool,
            tc.tile_pool(name="vb", bufs=1) as vb_pool,
            tc.tile_pool(name="wraw", bufs=2) as wraw,
            tc.tile_pool(name="wrnd", bufs=2) as wrnd,
            tc.tile_pool(name="expp", bufs=3) as expp,
            tc.tile_pool(name="small", bufs=2) as small,
            tc.tile_pool(name="small1", bufs=1) as small1,
            tc.tile_pool(name="ffs", bufs=2) as ffs,
            tc.tile_pool(name="dram", bufs=1, space="DRAM") as dram,
            tc.tile_pool(name="ps_a", bufs=4, space="PSUM") as ps_a,
            tc.tile_pool(name="ps_sc", bufs=2, space="PSUM") as ps_sc,
            tc.tile_pool(name="ps_ctx", bufs=2, space="PSUM") as ps_ctx,
        ):
            # ---- constants ----
            ident = consts.tile([P, P], dt.float32, tag="ident")
            make_identity(nc, ident[:])
            scr32 = consts.tile([P, QCH], dt.float32, tag="scr32")
            ones_r = consts.tile([P, P], MMDT, tag="ones")
            nc.vector.memset(scr32[:], 1.0)
            nc.vector.tensor_copy(out=ones_r[:], in_=scr32[:, 0:P])
            ones64 = consts.tile([P, HD], MMDT, tag="ones64")
            nc.vector.tensor_copy(out=ones64[:], in_=scr32[:, 0:HD])
            tri = consts.tile([P, QT_T, QCH], MMDT, tag="tri")
            for j in range(QT_T):
                # keep where f - p - 128j >= 0  <=>  (128j + p) <= f
                nc.vector.memset(scr32[:], 1.0)
                nc.gpsimd.affine_select(
                    out=scr32[:], in_=scr32[:],
                    compare_op=OP.is_ge, fill=0.0,
                    base=-P * j, pattern=[[1, QCH]], channel_multiplier=-1,
                )
                nc.vector.tensor_copy(out=tri[:, j, :], in_=scr32[:])
            kbias_sb = consts.tile([P, KT_TILES], dt.float32, tag="kbias")
            nc.sync.dma_start(kbias_sb[:], kbias[:])
            eps_sb = consts.tile([P, 1], dt.float32, tag="eps")
            nc.vector.memset(eps_sb[:], EPS)

            def load_vec_pd(name, ap, n):
                t = consts.tile([P, n], dt.float32, tag=name)
                nc.sync.dma_start(t[:], ap.rearrange("(o p) -> p o", p=P))
                return t

            bq_sb = load_vec_pd("bq", bq, NDT)
            bk_sb = load_vec_pd("bk", bk, NDT)
            bo_sb = load_vec_pd("bo", bo, NDT)
            b1_sb = load_vec_pd("b1", b1, NFT)
            b2_sb = load_vec_pd("b2", b2, NDT)
            g1_sb = load_vec_pd("g1", g1, NDT)
            be1_sb = load_vec_pd("be1", be1, NDT)
            g2_sb = load_vec_pd("g2", g2, NDT)
            be2_sb = load_vec_pd("be2", be2, NDT)
            bv_sb = consts.tile([P, D], dt.float32, tag="bv")
            nc.gpsimd.dma_start(out=bv_sb[:], in_=bv[None, :].to_broadcast([P, D]))
            colsum = consts.tile([P, 4, QCH], MMDT, tag="colsum")
            nc.vector.memset(scr32[:], 0.0)
            for _s in range(4):
                nc.vector.tensor_copy(out=colsum[:, _s, :], in_=scr32[:])

            ff_dram = dram.tile([NFT, P, QCH], MMDT)

            def transpose_in(src_ap, dst_tile, dst_do, dst_cols):
                """dst[:, dst_do, dst_cols] = (128x128 fp32 block).T via PE."""
                pt = ps_sc.tile([P, P], dt.float32, tag="ps_sc")
                nc.tensor.transpose(pt[:], src_ap, ident[:])
                nc.vector.tensor_copy(out=dst_tile[:, dst_do, dst_cols], in_=pt[:])

            def stream_round(dram_ap, shape, tag):
                """DMA a weight tile and round fp32 -> f32r on the Scalar
engine."""
                raw = wraw.tile(shape, dt.float32, tag=tag)
                nc.sync.dma_start(raw[:], dram_ap)
                rnd = wrnd.tile(shape, MMDT, tag=tag + "_r")
                nc.vector.tensor_copy(out=rnd[:], in_=raw[:])
                return rnd

            # ---- fused K/V projection + attention, chunk by chunk ----
            # (rotated order: chunk 0 IS the query chunk -> Q projected there)
            QT = mid.tile([P, NDT, QCH], MMDT, tag="mid", name="QT")
            xqT = mid.tile([P, NDT, QCH], MMDT, tag="mid", name="xqT")
            ctx_sb = mid.tile([P, NDT, QCH], dt.float32, tag="mid", name="ctx_sb")
            nc.vector.memset(ctx_sb[:], 0.0)
            for c in range(NCH):
                xTc = mid.tile([P, NDT, QCH], MMDT, tag="mid", name="xTc")
                for t in range(QT_T):
                    xn = small.tile([P, D], dt.float32, tag="xnat")
                    nc.sync.dma_start(xn[:], xb3[c, t])
                    for do in range(NDT):
                        transpose_in(xn[:, do * P:(do + 1) * P], xTc, do,
                                     slice(t * P, (t + 1) * P))
                if c == 0:
                    # query chunk: keep a copy for the residual, project Q
                    nc.vector.tensor_copy(out=xqT[:], in_=xTc[:])
                    for do in range(NDT):
                        wq_r = stream_round(WqT[:, :, do * P:(do + 1) * P],
                                            [P, NDT, P], "wsm")
                        pq = ps_a.tile([P, QCH], dt.float32, tag="ps_a")
                        for k in range(NDT):
                            nc.tensor.matmul(pq[:], wq_r[:, k, :], xTc[:, k, :],
                                             start=(k == 0), stop=(k == NDT - 1))
                        nc.vector.tensor_scalar(
                            out=QT[:, do, :], in0=pq[:],
                            scalar1=bq_sb[:, do:do + 1], scalar2=None, op0=OP.add)
                # K block: [d_out, 512 k-tokens]
                ktblk = ktb_pool.tile([P, NDT, QCH], MMDT, tag="ktb")
                for do in range(NDT):
                    wk_r = stream_round(WkT[:, :, do * P:(do + 1) * P],
                                        [P, NDT, P], "wsm")
                    pk = ps_a.tile([P, QCH], dt.float32, tag="ps_a")
                    for k in range(NDT):
                        nc.tensor.matmul(pk[:], wk_r[:, k, :], xTc[:, k, :],
                                         start=(k == 0), stop=(k == NDT - 1))
                    nc.vector.tensor_scalar(
                        out=ktblk[:, do, :], in0=pk[:],
                        scalar1=bk_sb[:, do:do + 1], scalar2=None, op0=OP.add)
                # V block: [tok, head, 64+1] with ones column
                vblk = vb_pool.tile([P, QT_T, H, HD + 1], MMDT, tag="vb")
                nc.vector.tensor_copy(out=vblk[:, :, :, HD], in_=ones64[:])
                for nh in range(2):
                    pvs = [ps_a.tile([P, QCH], dt.float32, tag="ps_a",
                                     name=f"pv{t}") for t in range(QT_T)]
                    for k in range(NDT):
                        wv_r = stream_round(
                            WvT[:, k, nh * QCH:(nh + 1) * QCH],
                            [P, QCH], "wv")
                        for t in range(QT_T):
                            nc.tensor.matmul(
                                pvs[t][:], xTc[:, k, t * P:(t + 1) * P],
                                wv_r[:],
                                start=(k == 0), stop=(k == NDT - 1))
                    for t in range(QT_T):
                        nc.vector.tensor_tensor(
                            vblk[:, t, nh * 8:(nh + 1) * 8, 0:HD],
                            pvs[t][:].rearrange("p (h d) -> p h d", d=HD),
                            bv_sb[:, nh * QCH:(nh + 1) * QCH].rearrange(
                                "p (h d) -> p h d", d=HD),
                            OP.add)
                if debug and c == 0:
                    nc.sync.dma_start(dbg["dbg_kt"][:], ktblk[:].bitcast(dt.float32))
                    nc.sync.dma_start(dbg["dbg_v"][:], vblk[:].bitcast(dt.float32))
                # attention: head pairs share a d-tile; the two K=64 score
                # matmuls go to disjoint PE row-strips (0-63 / 64-127) and
                # run concurrently via tile_position.
                for a in range(H // 2):
                    pcs = [ps_ctx.tile([P, QCH], dt.float32, tag="ps_ctx",
                                       name=f"pc{i}") for i in range(2)]
                    for j in range(QT_T):
                        ktg = c * QT_T + j
                        exs = []
                        for i in range(2):
                            bp = i * HD
                            psc = ps_sc.tile([P, QCH], dt.float32, tag="ps_sc",
                                             name=f"psc{i}")
                            nc.tensor.matmul(
                                psc[:], ktblk[bp:bp + HD, a, j * P:(j + 1) * P],
                                QT[bp:bp + HD, a, :], start=True, stop=True,
                                tile_position=(bp, 0))
                            ex = expp.tile([P, QCH], MMDT, tag="exp",
                                           name=f"ex{i}")
                            nc.scalar.activation(
                                out=ex[:], in_=psc[:], func=AF.Exp,
                                bias=kbias_sb[:, ktg:ktg + 1], scale=0.125)
                            if c == 0:
                                nc.vector.tensor_tensor(ex[:], ex[:],
                                                        tri[:, j, :], OP.mult)
                            exs.append(ex)
                        if debug and c == 0 and a == 0 and j == 0:
                            nc.sync.dma_start(dbg["dbg_ex"][:],
                                              exs[0][:].bitcast(dt.float32))
                        for i in range(2):
                            h = 2 * a + i
                            nc.tensor.matmul(
                                pcs[i][0:HD + 1, :], vblk[:, j, h, :], exs[i][:],
                                start=(j == 0), stop=(j == QT_T - 1))
                    for i in range(2):
                        h = 2 * a + i
                        bp = i * HD
                        nc.vector.tensor_tensor(
                            ctx_sb[bp:bp + HD, a, :], ctx_sb[bp:bp + HD, a, :],
                            pcs[i][0:HD, :], OP.add)
                        cb, cs = 32 * (h % 4), h // 4
                        nc.vector.tensor_tensor(
                            colsum[cb:cb + 1, cs, :], colsum[cb:cb + 1, cs, :],
                            pcs[i][HD:HD + 1, :], OP.add)

            if debug:
                nc.sync.dma_start(dbg["dbg_xqT"][:], xqT[:].bitcast(dt.float32))
                nc.sync.dma_start(dbg["dbg_QT"][:], QT[:].bitcast(dt.float32))
                nc.sync.dma_start(dbg["dbg_ctx"][:], ctx_sb[:])
                nc.sync.dma_start(dbg["dbg_cs"][:], colsum[:].bitcast(dt.float32))
            # normalize context -> f32r: reciprocal colsum, then broadcast
            # each head's row across partitions via a K=1 ones matmul in PSUM
            with nc.allow_low_precision(reason="f32r recip colsum, ~1e-4 ok"):
                for _s in range(4):
                    nc.vector.reciprocal(out=colsum[:, _s, :],
                                         in_=colsum[:, _s, :])
            ctxT = mid.tile([P, NDT, QCH], MMDT, tag="mid", name="ctxT")
            for h in range(H):
                dti, bp = h // 2, (h % 2) * HD
                cb, cs = 32 * (h % 4), h // 4
                prc = ps_sc.tile([P, QCH], dt.float32, tag="ps_sc")
                nc.tensor.matmul(prc[:], ones_r[cb:cb + 1, :],
                                 colsum[cb:cb + 1, cs, :], start=True, stop=True,
                                 tile_position=(cb, 0))
                nc.vector.tensor_tensor(
                    ctxT[bp:bp + HD, dti, :], ctx_sb[bp:bp + HD, dti, :],
                    prc[bp:bp + HD, :], OP.mult)

            # ---- O-proj + residual + LN1 ----
            yT = mid.tile([P, NDT, QCH], MMDT, tag="mid", name="yT")
            for do in range(NDT):
                wo_r = stream_round(WoT[:, :, do * P:(do + 1) * P],
                                    [P, NDT, P], "wsm")
                po = ps_a.tile([P, QCH], dt.float32, tag="ps_a")
                for k in range(NDT):
                    nc.tensor.matmul(po[:], wo_r[:, k, :], ctxT[:, k, :],
                                     start=(k == 0), stop=(k == NDT - 1))
                nc.vector.scalar_tensor_tensor(
                    out=yT[:, do, :], in0=po[:], scalar=bo_sb[:, do:do + 1],
                    in1=xqT[:, do, :], op0=OP.add, op1=OP.add)

            def layer_norm(src, dst, g_sb, be_sb):
                """dst[:, do, :] = LN(src) over d_model (partition + do axes);
                per-token (free-axis) stats via ones-matmul column sums."""
                ps1 = ps_a.tile([P, QCH], dt.float32, tag="ps_a")
                for do in range(NDT):
                    nc.tensor.matmul(ps1[:], ones_r[:], src[:, do, :],
                                     start=(do == 0), stop=(do == NDT - 1))
                ps2 = ps_a.tile([P, QCH], dt.float32, tag="ps_a")
                for do in range(NDT):
                    sq = small1.tile([P, QCH], MMDT, tag="sq")
                    nc.vector.tensor_tensor(sq[:], src[:, do, :], src[:, do, :],
                                            OP.mult)
                    nc.tensor.matmul(ps2[:], ones_r[:], sq[:],
                                     start=(do == 0), stop=(do == NDT - 1))
                mean = small1.tile([P, QCH], MMDT, tag="mean")
                nc.vector.tensor_scalar(out=mean[:], in0=ps1[:], scalar1=1.0 / D,
                                        scalar2=None, op0=OP.mult)
                m2 = small1.tile([P, QCH], MMDT, tag="m2")
                nc.vector.tensor_tensor(m2[:], mean[:], mean[:], OP.mult)
                var = small1.tile([P, QCH], MMDT, tag="var")
                nc.vector.scalar_tensor_tensor(
                    out=var[:], in0=ps2[:], scalar=1.0 / D, in1=m2[:],
                    op0=OP.mult, op1=OP.subtract)
                sstd = small1.tile([P, QCH], MMDT, tag="sstd")
                nc.scalar.activation(out=sstd[:], in_=var[:], func=AF.Sqrt,
                                     bias=eps_sb[:], scale=1.0)
                rstd = small1.tile([P, QCH], MMDT, tag="rstd")
                with nc.allow_low_precision(reason="f32r rstd, ~1e-4 rel ok"):
                    nc.vector.reciprocal(out=rstd[:], in_=sstd[:])
                for do in range(NDT):
                    t1 = small.tile([P, QCH], MMDT, tag="ln_t1")
                    nc.vector.tensor_tensor(t1[:], src[:, do, :], mean[:],
                                            OP.subtract)
                    nc.vector.tensor_tensor(t1[:], t1[:], rstd[:], OP.mult)
                    nc.vector.tensor_scalar(
                        out=dst[:, do, :], in0=t1[:],
                        scalar1=g_sb[:, do:do + 1], scalar2=be_sb[:, do:do + 1],
                        op0=OP.mult, op1=OP.add)

            hT = mid.tile([P, NDT, QCH], MMDT, tag="mid", name="hT")
            layer_norm(yT, hT, g1_sb, be1_sb)
            if debug:
                nc.sync.dma_start(dbg["dbg_ctxT"][:], ctxT[:].bitcast(dt.float32))
                nc.sync.dma_start(dbg["dbg_yT"][:], yT[:].bitcast(dt.float32))
                nc.sync.dma_start(dbg["dbg_hT"][:], hT[:].bitcast(dt.float32))

            # ---- FFN (d_ff intermediate bounces through DRAM) ----
            for ft in range(NFT):
                w1_r = stream_round(W1T[:, :, ft * P:(ft + 1) * P],
                                    [P, NDT, P], "wsm")
                pf = ps_a.tile([P, QCH], dt.float32, tag="ps_a")
                for k in range(NDT):
                    nc.tensor.matmul(pf[:], w1_r[:, k, :], hT[:, k, :],
                                     start=(k == 0), stop=(k == NDT - 1))
                ffo = ffs.tile([P, QCH], MMDT, tag="ffo")
                nc.scalar.activation(out=ffo[:], in_=pf[:], func=AF.Relu,
                                     bias=b1_sb[:, ft:ft + 1], scale=1.0)
                nc.sync.dma_start(ff_dram[ft], ffo[:])
            y2T = mid.tile([P, NDT, QCH], MMDT, tag="mid", name="y2T")
            for dog in range(2):
                pds = [(ps_a if d4 < 2 else ps_ctx).tile(
                    [P, QCH], dt.float32, tag=("ps_a" if d4 < 2 else "ps_ctx"),
                    name=f"pd{d4}") for d4 in range(4)]
                for k in range(NFT):
                    ffi = ffs.tile([P, QCH], MMDT, tag="ffi")
                    nc.sync.dma_start(ffi[:], ff_dram[k])
                    w2_r = stream_round(W2T[:, k, dog * QCH:(dog + 1) * QCH],
                                        [P, QCH], "wv")
                    for d4 in range(4):
                        nc.tensor.matmul(
                            pds[d4][:], w2_r[:, d4 * P:(d4 + 1) * P], ffi[:],
                            start=(k == 0), stop=(k == NFT - 1))
                for d4 in range(4):
                    do = dog * 4 + d4
                    nc.vector.scalar_tensor_tensor(
                        out=y2T[:, do, :], in0=pds[d4][:],
                        scalar=b2_sb[:, do:do + 1], in1=hT[:, do, :],
                        op0=OP.add, op1=OP.add)

            outT = mid.tile([P, NDT, QCH], dt.float32, tag="mid", name="outT")
            layer_norm(y2T, outT, g2_sb, be2_sb)

            # ---- transpose back, DMA out ----
            out3 = out.rearrange("(t p) d -> t p d", p=P)
            for t in range(QT_T):
                on = small.tile([P, D], dt.float32, tag="xnat")
                for do in range(NDT):
                    pt = ps_sc.tile([P, P], dt.float32, tag="ps_sc")
                    nc.tensor.transpose(pt[:], outT[:, do, t * P:(t + 1) * P],
                                        ident[:])
                    nc.vector.tensor_copy(out=on[:, do * P:(do + 1) * P], in_=pt[:])
                nc.sync.dma_start(out3[t], on[:])

    nc.finalize()
    return nc


def _get_nc(mm_dtype_name="float32r", debug=False):
    key = ("nc", mm_dtype_name, debug)
    if key not in _CACHE:
        _CACHE[key] = _build(mm_dtype_name, debug)
    return _CACHE[key]


def kernel(x, mask, Wq, bq, Wk, bk, Wv, bv, Wo, bo, W1, b1, W2, b2,
           gamma1, beta1, gamma2, beta2, _trace=False, _mm_dtype="float32r",
           _debug=False):
    from concourse.bass_utils import run_bass_kernel_spmd

    nc = _get_nc(_mm_dtype, _debug)
    x = np.ascontiguousarray(np.asarray(x, dtype=np.float32))
    shared = {
        "Wq": np.asarray(Wq, np.float32), "Wk": np.asarray(Wk, np.float32),
        "Wv": np.asarray(Wv, np.float32), "Wo": np.asarray(Wo, np.float32),
        "W1": np.asarray(W1, np.float32), "W2": np.asarray(W2, np.float32),
        "bq": np.asarray(bq, np.float32), "bk": np.asarray(bk, np.float32),
        "bv": np.asarray(bv, np.float32), "bo": np.asarray(bo, np.float32),
        "b1": np.asarray(b1, np.float32), "b2": np.asarray(b2, np.float32),
        "g1": np.asarray(gamma1, np.float32), "be1": np.asarray(beta1, np.float32),
        "g2": np.asarray(gamma2, np.float32), "be2": np.asarray(beta2, np.float32),
    }
    in_maps = []
    for c in range(NCORES):
        b, i = divmod(c, NCORES // B)
        q0 = i * QCH
        xb_rot = np.concatenate(
            [x[b, q0:q0 + QCH], x[b, :q0], x[b, q0 + QCH:]], axis=0)
        kb = np.zeros((P, KT_TILES), np.float32)
        n_ok = QT_T + q0 // P  # self tiles + past tiles
        kb[:, n_ok:] = NEG
        in_maps.append({
            "xb": np.ascontiguousarray(xb_rot),
            "kbias": kb,
            **shared,
        })
    res = run_bass_kernel_spmd(nc, in_maps, core_ids=list(range(NCORES)),
                               trace=_trace)
    out = np.empty((B, S, D), np.float32)
    for c in range(NCORES):
        b, i = divmod(c, NCORES // B)
        out[b, i * QCH:(i + 1) * QCH] = res.results[c]["out"]
    if _trace:
        _CACHE["last_result"] = res
    return out



# revision 6
# speedup vs baseline: 1.7609x; 1.7609x over previous
"""Decoder layer (attn + FFN + 2 layernorms) on 8 Trainium2 cores — v2.

Sharding: core c handles batch b = c//4, query chunk i = c%4 (512 tokens).
Each core redundantly computes K/V for the full sequence (communication-free).
Causality: key/value token order is rotated per core on the host (self chunk
first, then past, then future) so the mask structure is uniform across cores:
k-tiles 0-3 (self) get host-built triangular bf16 masks, the rest a per-core
additive bias (0 past, -1e30 future) folded into the softmax exp. Softmax is
unnormalized (scores O(+-8)); the denominator comes from a ones-column
appended to V and is divided out of the accumulated context.

v2 vs v1: everything bf16 (weights, x, activations) — converted and laid out
on the HOST, so no on-device f32r rounding passes and no PE transposes; K/V
computed once, flat, fully SBUF-resident; per-head context accumulates across
all 16 k-tiles directly in PSUM; the FFN intermediate (d_ff=4096, bf16) stays
in SBUF instead of bouncing through DRAM; PSUM->SBUF copies are spread across
Scalar/Vector/Pool engines.
"""

import sys

sys.path.insert(0, "/opt/trn_rl_repo")

import numpy as np

D = 1024          # d_model
H = 16            # heads
HD = 64           # head dim
DFF = 4096
EPS = 1e-6
B, S = 2, 2048
QCH = 512         # query tokens per core
NCORES = 8
P = 128
KT = S // P               # 16 k tiles of 128 tokens
NDT = D // P              # 8 d_model tiles
NFT = DFF // P            # 32 d_ff tiles
NEG = -1.0e30

_CACHE = {}


def _build(debug=False):
    import concourse.bacc as bacc
    import concourse.mybir as mybir
    import concourse.tile as tile

    dt = mybir.dt
    BF = dt.bfloat16
    AF = mybir.ActivationFunctionType
    OP = mybir.AluOpType

    nc = bacc.Bacc("TRN2", target_bir_lowering=False, debug=False)

    # ---- I/O (all host-pre-laid-out; bf16 for matmul operands) ----
    xT = nc.dram_tensor("xT", [P, NDT, S], BF, kind="ExternalInput")
    kbias = nc.dram_tensor("kbias", [P, KT], dt.float32, kind="ExternalInput")
    tri = nc.dram_tensor("tri", [P, 4, 2 * QCH], BF, kind="ExternalInput")
    onesr = nc.dram_tensor("onesr", [P, P], BF, kind="ExternalInput")
    wq = nc.dram_tensor("wq", [P, NDT, D], BF, kind="ExternalInput")
    wk = nc.dram_tensor("wk", [P, NDT, D], BF, kind="ExternalInput")
    wv = nc.dram_tensor("wv", [P, NDT, D], BF, kind="ExternalInput")
    wo = nc.dram_tensor("wo", [P, NDT, D], BF, kind="ExternalInput")
    w1 = nc.dram_tensor("w1", [P, NDT, DFF], BF, kind="ExternalInput")
    w2 = nc.dram_tensor("w2", [P, NFT, D], BF, kind="ExternalInput")
    bq = nc.dram_tensor("bq", [P, NDT], dt.float32, kind="ExternalInput")
    bk = nc.dram_tensor("bk", [P, NDT], dt.float32, kind="ExternalInput")
    bvb = nc.dram_tensor("bvb", [P, D], BF, kind="ExternalInput")
    bo = nc.dram_tensor("bo", [P, NDT], dt.float32, kind="ExternalInput")
    b1 = nc.dram_tensor("b1", [P, NFT], dt.float32, kind="ExternalInput")
    b2 = nc.dram_tensor("b2", [P, NDT], dt.float32, kind="ExternalInput")
    g1 = nc.dram_tensor("g1", [P, NDT], dt.float32, kind="ExternalInput")
    be1 = nc.dram_tensor("be1", [P, NDT], dt.float32, kind="ExternalInput")
    g2 = nc.dram_tensor("g2", [P, NDT], dt.float32, kind="ExternalInput")
    be2 = nc.dram_tensor("be2", [P, NDT], dt.float32, kind="ExternalInput")
    out = nc.dram_tensor("out", [P, NDT, QCH], dt.float32, kind="ExternalOutput")

    with tile.TileContext(nc) as tc:
        with (
            tc.tile_pool(name="consts", bufs=1) as consts,
            tc.tile_pool(name="wbig", bufs=2) as wbig,
            tc.tile_pool(name="mid", bufs=1) as mid,
            tc.tile_pool(name="expp", bufs=4) as expp,
            tc.tile_pool(name="small", bufs=2) as small,
        ):
            # ---- constants (all straight DMA) ----
            tri_sb = consts.tile([P, 4, 2 * QCH], BF, tag="tri")
            nc.sync.dma_start(tri_sb[:], tri[:])
            kbias_sb = consts.tile([P, KT], dt.float32, tag="kbias")
            nc.sync.dma_start(kbias_sb[:], kbias[:])
            onesr_sb = consts.tile([P, P], BF, tag="onesr")
            nc.sync.dma_start(onesr_sb[:], onesr[:])
            bvb_sb = consts.tile([P, D], BF, tag="bvb")
            nc.sync.dma_start(bvb_sb[:], bvb[:])
            eps_sb = consts.tile([P, 1], dt.float32, tag="eps")
            nc.vector.memset(eps_sb[:], EPS)

            def load_pd(name, ap, n):
                t = consts.tile([P, n], dt.float32, tag=name, name=name)
                nc.sync.dma_start(t[:], ap[:])
                return t

            bq_sb = load_pd("bq", bq, NDT)
            bk_sb = load_pd("bk", bk, NDT)
            bo_sb = load_pd("bo", bo, NDT)
            b1_sb = load_pd("b1", b1, NFT)
            b2_sb = load_pd("b2", b2, NDT)
            g1_sb = load_pd("g1", g1, NDT)
            be1_sb = load_pd("be1", be1, NDT)
            g2_sb = load_pd("g2", g2, NDT)
            be2_sb = load_pd("be2", be2, NDT)

            def wtile(src_ap, name):
                t = wbig.tile([P, NDT, D], BF, tag="w", name=name)
                nc.sync.dma_start(t[:], src_ap)
                return t

            def layer_norm(ps_pool, src, dst, g_sb, be_sb, dst_dt):
                """dst[:, do, :] = LN(src) over d_model (partition + do axes);
                per-token (free-axis) stats via ones-matmul column sums."""
                ps1 = ps_pool.tile([P, QCH], dt.float32, tag="ln", name="ps1")
                for do in range(NDT):
                    nc.tensor.matmul(ps1[:], onesr_sb[:], src[:, do, :],
                                     start=(do == 0), stop=(do == NDT - 1))
                ps2 = ps_pool.tile([P, QCH], dt.float32, tag="ln", name="ps2")
                for do in range(NDT):
                    sq = small.tile([P, QCH], BF, tag="sq")
                    nc.vector.tensor_tensor(sq[:], src[:, do, :], src[:, do, :],
                                            OP.mult)
                    nc.tensor.matmul(ps2[:], onesr_sb[:], sq[:],
                                     start=(do == 0), stop=(do == NDT - 1))
                mean = small.tile([P, QCH], BF, tag="mean")
                nc.vector.tensor_scalar(out=mean[:], in0=ps1[:], scalar1=1.0 / D,
                                        scalar2=None, op0=OP.mult)
                m2 = small.tile([P, QCH], BF, tag="m2")
                nc.vector.tensor_tensor(m2[:], mean[:], mean[:], OP.mult)
                var = small.tile([P, QCH], BF, tag="var")
                nc.vector.scalar_tensor_tensor(
                    out=var[:], in0=ps2[:], scalar=1.0 / D, in1=m2[:],
                    op0=OP.mult, op1=OP.subtract)
                sstd = small.tile([P, QCH], BF, tag="sstd")
                nc.scalar.activation(out=sstd[:], in_=var[:], func=AF.Sqrt,
                                     bias=eps_sb[:], scale=1.0)
                rstd = small.tile([P, QCH], BF, tag="rstd")
                with nc.allow_low_precision(reason="bf16 rstd, ~4e-3 rel ok"):
                    nc.vector.reciprocal(out=rstd[:], in_=sstd[:])
                for do in range(NDT):
                    t1 = small.tile([P, QCH], BF, tag="ln_t1")
                    nc.vector.tensor_tensor(t1[:], src[:, do, :], mean[:],
                                            OP.subtract)
                    nc.vector.tensor_tensor(t1[:], t1[:], rstd[:], OP.mult)
                    nc.vector.tensor_scalar(
                        out=dst[:, do, :], in0=t1[:],
                        scalar1=g_sb[:, do:do + 1], scalar2=be_sb[:, do:do + 1],
                        op0=OP.mult, op1=OP.add)

            ctxT = mid.tile([P, NDT, QCH], BF, tag="ctxT")
            yT = mid.tile([P, NDT, QCH], BF, tag="yT")
            hT = mid.tile([P, NDT, QCH], BF, tag="hT")

            with tc.tile_pool(name="attn", bufs=1) as attn:
                xT_sb = attn.tile([P, NDT, S], BF, tag="xT")
                nc.sync.dma_start(xT_sb[:], xT[:])
                kt_sb = attn.tile([P, NDT, S], BF, tag="kt")
                v_sb = attn.tile([P, KT, H, HD + 1], BF, tag="v")
                qT_sb = attn.tile([P, NDT, QCH], BF, tag="qT")
                nc.vector.memset(v_sb[:, :, :, HD], 1.0)

                # ---- projections ----
                with (
                    tc.tile_pool(name="psP", bufs=2, space="PSUM") as psP,
                ):
                    wq_t = wtile(wq[:], "wq_t")
                    for do in range(NDT):
                        pq = psP.tile([P, QCH], dt.float32, tag="pq")
                        for k in range(NDT):
                            nc.tensor.matmul(
                                pq[:], wq_t[:, k, do * P:(do + 1) * P],
                                xT_sb[:, k, 0:QCH],
                                start=(k == 0), stop=(k == NDT - 1))
                        nc.vector.tensor_scalar(
                            out=qT_sb[:, do, :], in0=pq[:],
                            scalar1=bq_sb[:, do:do + 1], scalar2=None,
                            op0=OP.add)
                    wk_t = wtile(wk[:], "wk_t")
                    for do in range(NDT):
                        for np_ in range(2):
                            pk = psP.tile([P, 2 * QCH], dt.float32, tag="pk")
                            for half in range(2):
                                n = 2 * np_ + half
                                for k in range(NDT):
                                    nc.tensor.matmul(
                                        pk[:, half * QCH:(half + 1) * QCH],
                                        wk_t[:, k, do * P:(do + 1) * P],
                                        xT_sb[:, k, n * QCH:(n + 1) * QCH],
                                        start=(k == 0), stop=(k == NDT - 1))
                            nc.scalar.activation(
                                out=kt_sb[:, do, np_ * 2 * QCH:(np_ + 1) * 2 * QCH],
                                in_=pk[:], func=AF.Identity,
                                bias=bk_sb[:, do:do + 1], scale=1.0)
                    wv_t = wtile(wv[:], "wv_t")
                    for tt in range(KT):
                        for nh in range(2):
                            pv = psP.tile([P, QCH], dt.float32, tag="pq",
                                          name="pv")
                            for k in range(NDT):
                                nc.tensor.matmul(
                                    pv[:], xT_sb[:, k, tt * P:(tt + 1) * P],
                                    wv_t[:, k, nh * QCH:(nh + 1) * QCH],
                                    start=(k == 0), stop=(k == NDT - 1))
                            nc.vector.tensor_tensor(
                                v_sb[:, tt, nh * 8:(nh + 1) * 8, 0:HD],
                                pv[:].rearrange("p (h d) -> p h d", d=HD),
                                bvb_sb[:, nh * QCH:(nh + 1) * QCH].rearrange(
                                    "p (h d) -> p h d", d=HD),
                                OP.add)

                # ---- attention: 8 head pairs, ctx accumulates in PSUM ----
                wo_t = wtile(wo[:], "wo_t")  # prefetch during attention
                with (
                    tc.tile_pool(name="psS", bufs=2, space="PSUM") as psS,
                    tc.tile_pool(name="psC", bufs=3, space="PSUM") as psC,
                ):
                    for a in range(H // 2):
                        pcs = [psC.tile([P, QCH], dt.float32, tag="pc",
                                        name=f"pc{i}") for i in range(2)]
                        for j in range(KT):
                            psc = psS.tile([P, 2 * QCH], dt.float32, tag="psc")
                            for i in range(2):
                                bp = i * HD
                                nc.tensor.matmul(
                                    psc[:, i * QCH:(i + 1) * QCH],
                                    kt_sb[bp:bp + HD, a, j * P:(j + 1) * P],
                                    qT_sb[bp:bp + HD, a, :],
                                    start=True, stop=True,
                                    tile_position=(bp, 0))
                            ex = expp.tile([P, 2 * QCH], BF, tag="exp")
                            nc.scalar.activation(
                                out=ex[:], in_=psc[:], func=AF.Exp,
                                bias=kbias_sb[:, j:j + 1], scale=0.125)
                            if j < 4:
                                nc.vector.tensor_tensor(ex[:], ex[:],
                                                        tri_sb[:, j, :],
                                                        OP.mult)
                            for i in range(2):
                                nc.tensor.matmul(
                                    pcs[i][0:HD + 1, :], v_sb[:, j, 2 * a + i, :],
                                    ex[:, i * QCH:(i + 1) * QCH],
                                    start=(j == 0), stop=(j == KT - 1))
                        # normalize: recip of ones-row, broadcast (Pool),
                        # multiply
                        for i in range(2):
                            rc = small.tile([1, QCH], BF, tag="rc", name="rc")
                            with nc.allow_low_precision(
                                    reason="bf16 recip colsum, ~4e-3 ok"):
                                nc.vector.reciprocal(out=rc[:],
                                                     in_=pcs[i][HD:HD + 1, :])
                            prcb = small.tile([HD, QCH], BF, tag="prcb",
                                              name="prcb")
                            nc.gpsimd.partition_broadcast(prcb[:], rc[:])
                            nc.vector.tensor_tensor(
                                ctxT[i * HD:(i + 1) * HD, a, :],
                                pcs[i][0:HD, :], prcb[:], OP.mult)

                # ---- O proj + residual + LN1 ----
                w1q = [None] * 4
                w1q[0] = wtile(w1[:, :, 0:D], "w1q")  # prefetch
                with tc.tile_pool(name="psO", bufs=2, space="PSUM") as psO:
                    for do in range(NDT):
                        po = psO.tile([P, QCH], dt.float32, tag="po")
                        for k in range(NDT):
                            nc.tensor.matmul(
                                po[:], wo_t[:, k, do * P:(do + 1) * P],
                                ctxT[:, k, :],
                                start=(k == 0), stop=(k == NDT - 1))
                        nc.vector.scalar_tensor_tensor(
                            out=yT[:, do, :], in0=po[:],
                            scalar=bo_sb[:, do:do + 1],
                            in1=xT_sb[:, do, 0:QCH], op0=OP.add, op1=OP.add)
                    layer_norm(psO, yT, hT, g1_sb, be1_sb, BF)

            # ---- FFN (intermediate stays in SBUF, bf16) ----
            with tc.tile_pool(name="ffnp", bufs=1) as ffnp:
                ff_sb = ffnp.tile([P, NFT, QCH], BF, tag="ff")
                y2T = ffnp.tile([P, NDT, QCH], BF, tag="y2T")
                outT = ffnp.tile([P, NDT, QCH], dt.float32, tag="outT")
                with tc.tile_pool(name="psF1", bufs=4, space="PSUM") as psF1:
                    for ft in range(NFT):
                        if ft % 8 == 0 and ft // 8 < 3:
                            q = ft // 8 + 1
                            w1q[q] = wtile(w1[:, :, q * D:(q + 1) * D], "w1q")
                        pf = psF1.tile([P, QCH], dt.float32, tag="pf")
                        wt = w1q[ft // 8]
                        for k in range(NDT):
                            nc.tensor.matmul(
                                pf[:], wt[:, k, (ft % 8) * P:(ft % 8 + 1) * P],
                                hT[:, k, :],
                                start=(k == 0), stop=(k == NDT - 1))
                        nc.scalar.activation(
                            out=ff_sb[:, ft, :], in_=pf[:], func=AF.Relu,
                            bias=b1_sb[:, ft:ft + 1], scale=1.0)
                with tc.tile_pool(name="psF2", bufs=8, space="PSUM") as psF2:
                    accs = [psF2.tile([P, QCH], dt.float32, tag="acc",
                                      name=f"acc{do}") for do in range(NDT)]
                    w2q = None
                    for k in range(NFT):
                        if k % 8 == 0:
                            w2q = wbig.tile([P, NDT, D], BF, tag="w",
                                            name="w2q")
                            nc.sync.dma_start(
                                w2q[:], w2[:, k:k + 8, :])
                        for do in range(NDT):
                            nc.tensor.matmul(
                                accs[do][:], w2q[:, k % 8, do * P:(do + 1) * P],
                                ff_sb[:, k, :],
                                start=(k == 0), stop=(k == NFT - 1))
                    for do in range(NDT):
                        nc.vector.scalar_tensor_tensor(
                            out=y2T[:, do, :], in0=accs[do][:],
                            scalar=b2_sb[:, do:do + 1], in1=hT[:, do, :],
                            op0=OP.add, op1=OP.add)
                with tc.tile_pool(name="psL2", bufs=2, space="PSUM") as psL2:
                    layer_norm(psL2, y2T, outT, g2_sb, be2_sb, dt.float32)
                nc.sync.dma_start(out[:], outT[:])

    nc.finalize()
    return nc


def _get_nc(debug=False):
    key = ("nc", debug)
    if key not in _CACHE:
        _CACHE[key] = _build(debug)
    return _CACHE[key]


def _prep_shared(Wq, bq, Wk, bk, Wv, bv, Wo, bo, W1, b1, W2, b2,
                 gamma1, beta1, gamma2, beta2):
    import ml_dtypes
    BF = ml_dtypes.bfloat16

    def wT(W):  # [D, N] -> [P, D//P, N] (d_in split over partitions)
        W = np.asarray(W, np.float32)
        kt = W.shape[0] // P
        return np.ascontiguousarray(
            W.reshape(kt, P, W.shape[1]).transpose(1, 0, 2).astype(BF))

    def pd(v):  # [N] -> [P, N//P] (per-partition layout, fp32)
        v = np.asarray(v, np.float32)
        return np.ascontiguousarray(v.reshape(-1, P).T)

    # triangular self-chunk masks, duplicated across the two 512 halves
    p_ = np.arange(P)[:, None]
    f_ = np.arange(QCH)[None, :]
    tri = np.zeros((P, 4, 2 * QCH), np.float32)
    for j in range(4):
        m = ((P * j + p_) <= f_).astype(np.float32)
        tri[:, j, 0:QCH] = m
        tri[:, j, QCH:] = m
    return {
        "wq": wT(Wq), "wk": wT(Wk), "wv": wT(Wv), "wo": wT(Wo),
        "w1": wT(W1), "w2": wT(W2),
        "bq": pd(bq), "bk": pd(bk), "bo": pd(bo),
        "b1": pd(b1), "b2": pd(b2),
        "g1": pd(gamma1), "be1": pd(beta1), "g2": pd(gamma2), "be2": pd(beta2),
        "bvb": np.ascontiguousarray(
            np.broadcast_to(np.asarray(bv, np.float32), (P, D)).astype(BF)),
        "tri": tri.astype(BF),
        "onesr": np.ones((P, P), BF),
    }


def kernel(x, mask, Wq, bq, Wk, bk, Wv, bv, Wo, bo, W1, b1, W2, b2,
           gamma1, beta1, gamma2, beta2, _trace=False, _debug=False,
           _mm_dtype=None):
    import ml_dtypes
    from concourse.bass_utils import run_bass_kernel_spmd

    BF = ml_dtypes.bfloat16
    nc = _get_nc(_debug)
    x = np.ascontiguousarray(np.asarray(x, dtype=np.float32))
    shared = _prep_shared(Wq, bq, Wk, bk, Wv, bv, Wo, bo, W1, b1, W2, b2,
                          gamma1, beta1, gamma2, beta2)
    in_maps = []
    for c in range(NCORES):
        b, i = divmod(c, NCORES // B)
        q0 = i * QCH
        xb_rot = np.concatenate(
            [x[b, q0:q0 + QCH], x[b, :q0], x[b, q0 + QCH:]], axis=0)
        # [S, D] -> [P, NDT, S] transposed layout, bf16
        xTc = xb_rot.T.reshape(NDT, P, S).transpose(1, 0, 2).astype(BF)
        kb = np.zeros((P, KT), np.float32)
        n_ok = 4 + q0 // P  # self tiles + past tiles
        kb[:, n_ok:] = NEG
        in_maps.append({
            "xT": np.ascontiguousarray(xTc),
            "kbias": kb,
            **shared,
        })
    res = run_bass_kernel_spmd(nc, in_maps, core_ids=list(range(NCORES)),
                               trace=_trace)
    out = np.empty((B, S, D), np.float32)
    for c in range(NCORES):
        b, i = divmod(c, NCORES // B)
        r = np.asarray(res.results[c]["out"], np.float32)  # [P, NDT, QCH]
        out[b, i * QCH:(i + 1) * QCH] = (
            r.transpose(2, 1, 0).reshape(QCH, D))
    if _trace:
        _CACHE["last_result"] = res
    return out


# revision 14
# speedup vs baseline: 1.8467x; 1.0487x over previous
"""Decoder layer (attn + FFN + 2 layernorms) on 8 Trainium2 cores — v2.

Sharding: core c handles batch b = c//4, query chunk i = c%4 (512 tokens).
Each core redundantly computes K/V for the full sequence (communication-free).
Causality: key/value token order is rotated per core on the host (self chunk
first, then past, then future) so the mask structure is uniform across cores:
k-tiles 0-3 (self) get host-built triangular bf16 masks, the rest a per-core
additive bias (0 past, -1e30 future) folded into the softmax exp. Softmax is
unnormalized (scores O(+-8)); the denominator comes from a ones-column
appended to V and is divided out of the accumulated context.

v2 vs v1: everything bf16 (weights, x, activations) — converted and laid out
on the HOST, so no on-device f32r rounding passes and no PE transposes; K/V
computed once, flat, fully SBUF-resident; per-head context accumulates across
all 16 k-tiles directly in PSUM; the FFN intermediate (d_ff=4096, bf16) stays
in SBUF instead of bouncing through DRAM; PSUM->SBUF copies are spread across
Scalar/Vector/Pool engines.
"""

import sys

sys.path.insert(0, "/opt/trn_rl_repo")

import numpy as np

D = 1024          # d_model
H = 16            # heads
HD = 64           # head dim
DFF = 4096
EPS = 1e-6
B, S = 2, 2048
QCH = 512         # query tokens per core
NCORES = 8
P = 128
KT = S // P               # 16 k tiles of 128 tokens
NDT = D // P              # 8 d_model tiles
NFT = DFF // P            # 32 d_ff tiles
NEG = -1.0e30

_CACHE = {}


def _build(debug=False):
    import concourse.bacc as bacc
    import concourse.mybir as mybir
    import concourse.tile as tile

    dt = mybir.dt
    BF = dt.bfloat16
    AF = mybir.ActivationFunctionType
    OP = mybir.AluOpType

    nc = bacc.Bacc("TRN2", target_bir_lowering=False, debug=False)

    # ---- I/O (all host-pre-laid-out; bf16 for matmul operands) ----
    xT = nc.dram_tensor("xT", [P, NDT, S], BF, kind="ExternalInput")
    kbias = nc.dram_tensor("kbias", [P, KT], dt.float32, kind="ExternalInput")
    tri = nc.dram_tensor("tri", [P, 4, 2 * QCH], BF, kind="ExternalInput")
    onesr = nc.dram_tensor("onesr", [P, P], BF, kind="ExternalInput")
    wq = nc.dram_tensor("wq", [P, NDT, D], BF, kind="ExternalInput")
    wk = nc.dram_tensor("wk", [P, NDT, D], BF, kind="ExternalInput")
    wv = nc.dram_tensor("wv", [P, NDT, D], BF, kind="ExternalInput")
    wo = nc.dram_tensor("wo", [P, NDT, D], BF, kind="ExternalInput")
    w1 = nc.dram_tensor("w1", [P, NDT, DFF], BF, kind="ExternalInput")
    w2 = nc.dram_tensor("w2", [P, NFT, D], BF, kind="ExternalInput")
    bq = nc.dram_tensor("bq", [P, NDT], dt.float32, kind="ExternalInput")
    bk = nc.dram_tensor("bk", [P, NDT], dt.float32, kind="ExternalInput")
    bvb = nc.dram_tensor("bvb", [P, D], BF, kind="ExternalInput")
    bo = nc.dram_tensor("bo", [P, NDT], dt.float32, kind="ExternalInput")
    b1 = nc.dram_tensor("b1", [P, NFT], dt.float32, kind="ExternalInput")
    b2 = nc.dram_tensor("b2", [P, NDT], dt.float32, kind="ExternalInput")
    g1 = nc.dram_tensor("g1", [P, NDT], dt.float32, kind="ExternalInput")
    be1 = nc.dram_tensor("be1", [P, NDT], dt.float32, kind="ExternalInput")
    g2 = nc.dram_tensor("g2", [P, NDT], dt.float32, kind="ExternalInput")
    be2 = nc.dram_tensor("be2", [P, NDT], dt.float32, kind="ExternalInput")
    out = nc.dram_tensor("out", [P, NDT, QCH], dt.float32, kind="ExternalOutput")

    with tile.TileContext(nc) as tc:
        with (
            tc.tile_pool(name="consts", bufs=1) as consts,
            tc.tile_pool(name="wbig", bufs=2) as wbig,
            tc.tile_pool(name="mid", bufs=1) as mid,
            tc.tile_pool(name="expp", bufs=4) as expp,
            tc.tile_pool(name="small", bufs=2) as small,
        ):
            # ---- constants (all straight DMA) ----
            tri_sb = consts.tile([P, 4, 2 * QCH], BF, tag="tri")
            nc.sync.dma_start(tri_sb[:], tri[:])
            kbias_sb = consts.tile([P, KT], dt.float32, tag="kbias")
            nc.sync.dma_start(kbias_sb[:], kbias[:])
            onesr_sb = consts.tile([P, P], BF, tag="onesr")
            nc.sync.dma_start(onesr_sb[:], onesr[:])
            bvb_sb = consts.tile([P, D], BF, tag="bvb")
            nc.sync.dma_start(bvb_sb[:], bvb[:])
            eps_sb = consts.tile([P, 1], dt.float32, tag="eps")
            nc.vector.memset(eps_sb[:], EPS)

            def load_pd(name, ap, n):
                t = consts.tile([P, n], dt.float32, tag=name, name=name)
                nc.sync.dma_start(t[:], ap[:])
                return t

            bq_sb = load_pd("bq", bq, NDT)
            bk_sb = load_pd("bk", bk, NDT)
            bo_sb = load_pd("bo", bo, NDT)
            b1_sb = load_pd("b1", b1, NFT)
            b2_sb = load_pd("b2", b2, NDT)
            g1_sb = load_pd("g1", g1, NDT)
            be1_sb = load_pd("be1", be1, NDT)
            g2_sb = load_pd("g2", g2, NDT)
            be2_sb = load_pd("be2", be2, NDT)

            def wtile(src_ap, name):
                t = wbig.tile([P, NDT, D], BF, tag="w", name=name)
                nc.sync.dma_start(t[:], src_ap)
                return t

            def layer_norm(ps_pool, src, dst, g_sb, be_sb, out_dma=None):
                """dst[:, do, :] = LN(src) over d_model (partition + do axes);
                per-token (free-axis) stats via ones-matmul column sums."""
                ps1 = ps_pool.tile([P, QCH], dt.float32, tag="ln", name="ps1")
                for do in range(NDT):
                    nc.tensor.matmul(ps1[:], onesr_sb[:], src[:, do, :],
                                     start=(do == 0), stop=(do == NDT - 1))
                ps2 = ps_pool.tile([P, QCH], dt.float32, tag="ln", name="ps2")
                for do in range(NDT):
                    sq = small.tile([P, QCH], BF, tag="sq")
                    nc.vector.tensor_tensor(sq[:], src[:, do, :], src[:, do, :],
                                            OP.mult)
                    nc.tensor.matmul(ps2[:], onesr_sb[:], sq[:],
                                     start=(do == 0), stop=(do == NDT - 1))
                mean = small.tile([P, QCH], BF, tag="mean")
                nc.vector.tensor_scalar(out=mean[:], in0=ps1[:], scalar1=1.0 / D,
                                        scalar2=None, op0=OP.mult)
                m2 = small.tile([P, QCH], BF, tag="m2")
                nc.vector.tensor_tensor(m2[:], mean[:], mean[:], OP.mult)
                var = small.tile([P, QCH], BF, tag="var")
                nc.vector.scalar_tensor_tensor(
                    out=var[:], in0=ps2[:], scalar=1.0 / D, in1=m2[:],
                    op0=OP.mult, op1=OP.subtract)
                sstd = small.tile([P, QCH], BF, tag="sstd")
                nc.scalar.activation(out=sstd[:], in_=var[:], func=AF.Sqrt,
                                     bias=eps_sb[:], scale=1.0)
                rstd = small.tile([P, QCH], BF, tag="rstd")
                with nc.allow_low_precision(reason="bf16 rstd, ~4e-3 rel ok"):
                    nc.vector.reciprocal(out=rstd[:], in_=sstd[:])
                for do in range(NDT):
                    t1 = small.tile([P, QCH], BF, tag="ln_t1")
                    nc.vector.tensor_tensor(t1[:], src[:, do, :], mean[:],
                                            OP.subtract)
                    nc.vector.tensor_tensor(t1[:], t1[:], rstd[:], OP.mult)
                    nc.vector.tensor_scalar(
                        out=dst[:, do, :], in0=t1[:],
                        scalar1=g_sb[:, do:do + 1], scalar2=be_sb[:, do:do + 1],
                        op0=OP.mult, op1=OP.add)
                    if out_dma is not None:
                        nc.sync.dma_start(out_dma[:, do, :], dst[:, do, :])

            ctxT = mid.tile([P, NDT, QCH], BF, tag="ctxT")
            yT = mid.tile([P, NDT, QCH], BF, tag="yT")
            hT = mid.tile([P, NDT, QCH], BF, tag="hT")

            with tc.tile_pool(name="attn", bufs=1) as attn:
                xT_sb = attn.tile([P, NDT, S], BF, tag="xT")
                nc.sync.dma_start(xT_sb[:, :, 0:QCH], xT[:, :, 0:QCH])
                nc.sync.dma_start(xT_sb[:, :, QCH:], xT[:, :, QCH:])
                kt_sb = attn.tile([P, NDT, S], BF, tag="kt")
                v_sb = attn.tile([P, KT, H, HD + 1], BF, tag="v")
                qT_sb = attn.tile([P, NDT, QCH], BF, tag="qT")
                nc.vector.memset(v_sb[:, :, :, HD], 1.0)

                # ---- projections ----
                with (
                    tc.tile_pool(name="psP", bufs=2, space="PSUM") as psP,
                ):
                    wq_t = wtile(wq[:], "wq_t")
                    for do in range(NDT):
                        pq = psP.tile([P, QCH], dt.float32, tag="pq")
                        for k in range(NDT):
                            nc.tensor.matmul(
                                pq[:], wq_t[:, k, do * P:(do + 1) * P],
                                xT_sb[:, k, 0:QCH],
                                start=(k == 0), stop=(k == NDT - 1))
                        nc.vector.tensor_scalar(
                            out=qT_sb[:, do, :], in0=pq[:],
                            scalar1=bq_sb[:, do:do + 1], scalar2=None,
                            op0=OP.add)
                    wk_t = wtile(wk[:], "wk_t")
                    for do in range(NDT):
                        for np_ in range(2):
                            pk = psP.tile([P, 2 * QCH], dt.float32, tag="pk")
                            for half in range(2):
                                n = 2 * np_ + half
                                for k in range(NDT):
                                    nc.tensor.matmul(
                                        pk[:, half * QCH:(half + 1) * QCH],
                                        wk_t[:, k, do * P:(do + 1) * P],
                                        xT_sb[:, k, n * QCH:(n + 1) * QCH],
                                        start=(k == 0), stop=(k == NDT - 1))
                            nc.scalar.activation(
                                out=kt_sb[:, do, np_ * 2 * QCH:(np_ + 1) * 2 * QCH],
                                in_=pk[:], func=AF.Identity,
                                bias=bk_sb[:, do:do + 1], scale=1.0)
                    wv_t = wtile(wv[:], "wv_t")
                    for tt in range(KT):
                        pv = psP.tile([P, QCH], dt.float32, tag="pq",
                                      name="pv")
                        for k in range(NDT):
                            nc.tensor.matmul(
                                pv[:], xT_sb[:, k, tt * P:(tt + 1) * P],
                                wv_t[:, k, 0:QCH],
                                start=(k == 0), stop=(k == NDT - 1))
                        nc.vector.tensor_tensor(
                            v_sb[:, tt, 0:8, 0:HD],
                            pv[:].rearrange("p (h d) -> p h d", d=HD),
                            bvb_sb[:, 0:QCH].rearrange(
                                "p (h d) -> p h d", d=HD),
                            OP.add)

                # ---- attention: 8 head pairs, ctx accumulates in PSUM ----
                wo_t = wtile(wo[:], "wo_t")  # prefetch during attention
                with (
                    tc.tile_pool(name="psS", bufs=2, space="PSUM") as psS,
                    tc.tile_pool(name="psC", bufs=3, space="PSUM") as psC,
                ):
                    for a in range(H // 2):
                        pcs = [psC.tile([P, QCH], dt.float32, tag="pc",
                                        name=f"pc{i}") for i in range(2)]

                        def emit_scores(j, a=a):
                            psc = psS.tile([P, 2 * QCH], dt.float32,
                                           tag="psc", name="psc")
                            for i in range(2):
                                bp = i * HD
                                nc.tensor.matmul(
                                    psc[:, i * QCH:(i + 1) * QCH],
                                    kt_sb[bp:bp + HD, a, j * P:(j + 1) * P],
                                    qT_sb[bp:bp + HD, a, :],
                                    start=True, stop=True,
                                    tile_position=(bp, 0))
                            return psc

                        psc_cur = emit_scores(0)
                        for j in range(KT):
                            # software pipeline: next j's scores go to the PE
                            # ahead of this j's ctx so the PE never waits on
                            # the Scalar engine's exp
                            psc_next = emit_scores(j + 1) if j + 1 < KT else None
                            ex = expp.tile([P, 2 * QCH], BF, tag="exp")
                            nc.scalar.activation(
                                out=ex[:], in_=psc_cur[:], func=AF.Exp,
                                bias=kbias_sb[:, j:j + 1], scale=0.125)
                            if j < 4:
                                nc.vector.tensor_tensor(ex[:], ex[:],
                                                        tri_sb[:, j, :],
                                                        OP.mult)
                            for i in range(2):
                                nc.tensor.matmul(
                                    pcs[i][0:HD + 1, :], v_sb[:, j, 2 * a + i, :],
                                    ex[:, i * QCH:(i + 1) * QCH],
                                    start=(j == 0), stop=(j == KT - 1))
                            psc_cur = psc_next
                            # second half of the V projection (heads 8-15),
                            # interleaved into the PE stream of pairs 0-3
                            if a < 4 and j % 4 == 3:
                                tt = 4 * a + j // 4
                                pv1 = psC.tile([P, QCH], dt.float32, tag="pv",
                                               bufs=1, name="pv1")
                                for k in range(NDT):
                                    nc.tensor.matmul(
                                        pv1[:], xT_sb[:, k, tt * P:(tt + 1) * P],
                                        wv_t[:, k, QCH:2 * QCH],
                                        start=(k == 0), stop=(k == NDT - 1))
                                nc.vector.tensor_tensor(
                                    v_sb[:, tt, 8:16, 0:HD],
                                    pv1[:].rearrange("p (h d) -> p h d", d=HD),
                                    bvb_sb[:, QCH:2 * QCH].rearrange(
                                        "p (h d) -> p h d", d=HD),
                                    OP.add)
                        # normalize: recip of ones-row, broadcast (Pool),
                        # multiply
                        for i in range(2):
                            rc = small.tile([1, QCH], BF, tag="rc", name="rc")
                            with nc.allow_low_precision(
                                    reason="bf16 recip colsum, ~4e-3 ok"):
                                nc.vector.reciprocal(out=rc[:],
                                                     in_=pcs[i][HD:HD + 1, :])
                            prcb = small.tile([HD, QCH], BF, tag="prcb",
                                              name="prcb")
                            nc.gpsimd.partition_broadcast(prcb[:], rc[:])
                            nc.vector.tensor_tensor(
                                ctxT[i * HD:(i + 1) * HD, a, :],
                                pcs[i][0:HD, :], prcb[:], OP.mult)

                # ---- O proj + residual + LN1 ----
                w1q = [None] * 4
                w1q[0] = wtile(w1[:, :, 0:D], "w1q")  # prefetch
                with tc.tile_pool(name="psO", bufs=2, space="PSUM") as psO:
                    for do in range(NDT):
                        po = psO.tile([P, QCH], dt.float32, tag="po")
                        for k in range(NDT):
                            nc.tensor.matmul(
                                po[:], wo_t[:, k, do * P:(do + 1) * P],
                                ctxT[:, k, :],
                                start=(k == 0), stop=(k == NDT - 1))
                        nc.vector.scalar_tensor_tensor(
                            out=yT[:, do, :], in0=po[:],
                            scalar=bo_sb[:, do:do + 1],
                            in1=xT_sb[:, do, 0:QCH], op0=OP.add, op1=OP.add)
                    layer_norm(psO, yT, hT, g1_sb, be1_sb)

            # ---- FFN (intermediate stays in SBUF, bf16) ----
            with tc.tile_pool(name="ffnp", bufs=1) as ffnp:
                ff_sb = ffnp.tile([P, NFT, QCH], BF, tag="ff")
                y2T = ffnp.tile([P, NDT, QCH], BF, tag="y2T")
                outT = ffnp.tile([P, NDT, QCH], dt.float32, tag="outT")
                w2qs = {}
                with tc.tile_pool(name="psF1", bufs=4, space="PSUM") as psF1:
                    for ft in range(NFT):
                        if ft % 8 == 0 and ft // 8 < 3:
                            q = ft // 8 + 1
                            w1q[q] = wtile(w1[:, :, q * D:(q + 1) * D], "w1q")
                        if ft == 16:  # prefetch first W2 quarter
                            w2qs[0] = wbig.tile([P, NDT, D], BF, tag="w",
                                                name="w2q")
                            nc.sync.dma_start(w2qs[0][:], w2[:, 0:8, :])
                        pf = psF1.tile([P, QCH], dt.float32, tag="pf")
                        wt = w1q[ft // 8]
                        for k in range(NDT):
                            nc.tensor.matmul(
                                pf[:], wt[:, k, (ft % 8) * P:(ft % 8 + 1) * P],
                                hT[:, k, :],
                                start=(k == 0), stop=(k == NDT - 1))
                        nc.scalar.activation(
                            out=ff_sb[:, ft, :], in_=pf[:], func=AF.Relu,
                            bias=b1_sb[:, ft:ft + 1], scale=1.0)
                with tc.tile_pool(name="psF2", bufs=8, space="PSUM") as psF2:
                    accs = [psF2.tile([P, QCH], dt.float32, tag="acc",
                                      name=f"acc{do}") for do in range(NDT)]
                    for k in range(NFT):
                        if k % 8 == 0 and k > 0:
                            w2qs[k // 8] = wbig.tile([P, NDT, D], BF, tag="w",
                                                     name="w2q")
                            nc.sync.dma_start(w2qs[k // 8][:],
                                              w2[:, k:k + 8, :])
                        w2q = w2qs[k // 8]
                        for do in range(NDT):
                            nc.tensor.matmul(
                                accs[do][:], w2q[:, k % 8, do * P:(do + 1) * P],
                                ff_sb[:, k, :],
                                start=(k == 0), stop=(k == NFT - 1))
                    for do in range(NDT):
                        nc.vector.scalar_tensor_tensor(
                            out=y2T[:, do, :], in0=accs[do][:],
                            scalar=b2_sb[:, do:do + 1], in1=hT[:, do, :],
                            op0=OP.add, op1=OP.add)
                with tc.tile_pool(name="psL2", bufs=2, space="PSUM") as psL2:
                    layer_norm(psL2, y2T, outT, g2_sb, be2_sb, out_dma=out)

    nc.finalize()
    return nc


def _get_nc(debug=False):
    key = ("nc", debug)
    if key not in _CACHE:
        _CACHE[key] = _build(debug)
    return _CACHE[key]


def _prep_shared(Wq, bq, Wk, bk, Wv, bv, Wo, bo, W1, b1, W2, b2,
                 gamma1, beta1, gamma2, beta2):
    import ml_dtypes
    BF = ml_dtypes.bfloat16

    def wT(W):  # [D, N] -> [P, D//P, N] (d_in split over partitions)
        W = np.asarray(W, np.float32)
        kt = W.shape[0] // P
        return np.ascontiguousarray(
            W.reshape(kt, P, W.shape[1]).transpose(1, 0, 2).astype(BF))

    def pd(v):  # [N] -> [P, N//P] (per-partition layout, fp32)
        v = np.asarray(v, np.float32)
        return np.ascontiguousarray(v.reshape(-1, P).T)

    # triangular self-chunk masks, duplicated across the two 512 halves
    p_ = np.arange(P)[:, None]
    f_ = np.arange(QCH)[None, :]
    tri = np.zeros((P, 4, 2 * QCH), np.float32)
    for j in range(4):
        m = ((P * j + p_) <= f_).astype(np.float32)
        tri[:, j, 0:QCH] = m
        tri[:, j, QCH:] = m
    return {
        "wq": wT(Wq), "wk": wT(Wk), "wv": wT(Wv), "wo": wT(Wo),
        "w1": wT(W1), "w2": wT(W2),
        "bq": pd(bq), "bk": pd(bk), "bo": pd(bo),
        "b1": pd(b1), "b2": pd(b2),
        "g1": pd(gamma1), "be1": pd(beta1), "g2": pd(gamma2), "be2": pd(beta2),
        "bvb": np.ascontiguousarray(
            np.broadcast_to(np.asarray(bv, np.float32), (P, D)).astype(BF)),
        "tri": tri.astype(BF),
        "onesr": np.ones((P, P), BF),
    }


def kernel(x, mask, Wq, bq, Wk, bk, Wv, bv, Wo, bo, W1, b1, W2, b2,
           gamma1, beta1, gamma2, beta2, _trace=False, _debug=False,
           _mm_dtype=None):
    import ml_dtypes
    from concourse.bass_utils import run_bass_kernel_spmd

    BF = ml_dtypes.bfloat16
    nc = _get_nc(_debug)
    x = np.ascontiguousarray(np.asarray(x, dtype=np.float32))
    shared = _prep_shared(Wq, bq, Wk, bk, Wv, bv, Wo, bo, W1, b1, W2, b2,
                          gamma1, beta1, gamma2, beta2)
    in_maps = []
    for c in range(NCORES):
        b, i = divmod(c, NCORES // B)
        q0 = i * QCH
        xb_rot = np.concatenate(
            [x[b, q0:q0 + QCH], x[b, :q0], x[b, q0 + QCH:]], axis=0)
        # [S, D] -> [P, NDT, S] transposed layout, bf16
        xTc = xb_rot.T.reshape(NDT, P, S).transpose(1, 0, 2).astype(BF)
        kb = np.zeros((P, KT), np.float32)
        n_ok = 4 + q0 // P  # self tiles + past tiles
        kb[:, n_ok:] = NEG
        in_maps.append({
            "xT": np.ascontiguousarray(xTc),
            "kbias": kb,
            **shared,
        })
    res = run_bass_kernel_spmd(nc, in_maps, core_ids=list(range(NCORES)),
                               trace=_trace)
    out = np.empty((B, S, D), np.float32)
    for c in range(NCORES):
        b, i = divmod(c, NCORES // B)
        r = np.asarray(res.results[c]["out"], np.float32)  # [P, NDT, QCH]
        out[b, i * QCH:(i + 1) * QCH] = (
            r.transpose(2, 1, 0).reshape(QCH, D))
    if _trace:
        _CACHE["last_result"] = res
    return out


# revision 22
# speedup vs baseline: 1.8698x; 1.0126x over previous
"""Decoder layer (attn + FFN + 2 layernorms) on 8 Trainium2 cores — v2.

Sharding: core c handles batch b = c//4, query chunk i = c%4 (512 tokens).
Each core redundantly computes K/V for the full sequence (communication-free).
Causality: key/value token order is rotated per core on the host (self chunk
first, then past, then future) so the mask structure is uniform across cores:
k-tiles 0-3 (self) get host-built triangular bf16 masks, the rest a per-core
additive bias (0 past, -1e30 future) folded into the softmax exp. Softmax is
unnormalized (scores O(+-8)); the denominator comes from a ones-column
appended to V and is divided out of the accumulated context.

v2 vs v1: everything bf16 (weights, x, activations) — converted and laid out
on the HOST, so no on-device f32r rounding passes and no PE transposes; K/V
computed once, flat, fully SBUF-resident; per-head context accumulates across
all 16 k-tiles directly in PSUM; the FFN intermediate (d_ff=4096, bf16) stays
in SBUF instead of bouncing through DRAM; PSUM->SBUF copies are spread across
Scalar/Vector/Pool engines.
"""

import sys

sys.path.insert(0, "/opt/trn_rl_repo")

import numpy as np

D = 1024          # d_model
H = 16            # heads
HD = 64           # head dim
DFF = 4096
EPS = 1e-6
B, S = 2, 2048
QCH = 512         # query tokens per core
NCORES = 8
P = 128
KT = S // P               # 16 k tiles of 128 tokens
NDT = D // P              # 8 d_model tiles
NFT = DFF // P            # 32 d_ff tiles
NEG = -1.0e30

_CACHE = {}


def _build(debug=False):
    import concourse.bacc as bacc
    import concourse.mybir as mybir
    import concourse.tile as tile

    dt = mybir.dt
    BF = dt.bfloat16
    AF = mybir.ActivationFunctionType
    OP = mybir.AluOpType

    nc = bacc.Bacc("TRN2", target_bir_lowering=False, debug=False)

    # ---- I/O (all host-pre-laid-out; bf16 for matmul operands) ----
    xT = nc.dram_tensor("xT", [P, NDT, S], BF, kind="ExternalInput")
    kbias = nc.dram_tensor("kbias", [P, KT], dt.float32, kind="ExternalInput")
    tri = nc.dram_tensor("tri", [P, 4, 2 * QCH], BF, kind="ExternalInput")
    onesr = nc.dram_tensor("onesr", [P, P], BF, kind="ExternalInput")
    wq = nc.dram_tensor("wq", [P, NDT, D], BF, kind="ExternalInput")
    wk = nc.dram_tensor("wk", [P, NDT, D], BF, kind="ExternalInput")
    wv = nc.dram_tensor("wv", [P, NDT, D], BF, kind="ExternalInput")
    wo = nc.dram_tensor("wo", [P, NDT, D], BF, kind="ExternalInput")
    w1 = nc.dram_tensor("w1", [P, NDT, DFF], BF, kind="ExternalInput")
    w2 = nc.dram_tensor("w2", [P, NFT, D], BF, kind="ExternalInput")
    bq = nc.dram_tensor("bq", [P, NDT], dt.float32, kind="ExternalInput")
    bk = nc.dram_tensor("bk", [P, NDT], dt.float32, kind="ExternalInput")
    bvb = nc.dram_tensor("bvb", [P, D], BF, kind="ExternalInput")
    bo = nc.dram_tensor("bo", [P, NDT], dt.float32, kind="ExternalInput")
    b1 = nc.dram_tensor("b1", [P, NFT], dt.float32, kind="ExternalInput")
    b2 = nc.dram_tensor("b2", [P, NDT], dt.float32, kind="ExternalInput")
    g1 = nc.dram_tensor("g1", [P, NDT], dt.float32, kind="ExternalInput")
    be1 = nc.dram_tensor("be1", [P, NDT], dt.float32, kind="ExternalInput")
    g2 = nc.dram_tensor("g2", [P, NDT], dt.float32, kind="ExternalInput")
    be2 = nc.dram_tensor("be2", [P, NDT], dt.float32, kind="ExternalInput")
    out = nc.dram_tensor("out", [P, NDT, QCH], dt.float32, kind="ExternalOutput")

    with tile.TileContext(nc) as tc:
        with (
            tc.tile_pool(name="consts", bufs=1) as consts,
            tc.tile_pool(name="wbig", bufs=2) as wbig,
            tc.tile_pool(name="mid", bufs=1) as mid,
            tc.tile_pool(name="expp", bufs=4) as expp,
            tc.tile_pool(name="small", bufs=2) as small,
        ):
            # ---- constants (bulk ones deferred below the gating DMAs) ----
            kbias_sb = consts.tile([P, KT], dt.float32, tag="kbias")
            nc.sync.dma_start(kbias_sb[:], kbias[:])
            eps_sb = consts.tile([P, 1], dt.float32, tag="eps")
            nc.vector.memset(eps_sb[:], EPS)

            def load_pd(name, ap, n):
                t = consts.tile([P, n], dt.float32, tag=name, name=name)
                nc.sync.dma_start(t[:], ap[:])
                return t

            bq_sb = load_pd("bq", bq, NDT)
            bk_sb = load_pd("bk", bk, NDT)
            bo_sb = load_pd("bo", bo, NDT)
            b1_sb = load_pd("b1", b1, NFT)
            b2_sb = load_pd("b2", b2, NDT)
            g1_sb = load_pd("g1", g1, NDT)
            be1_sb = load_pd("be1", be1, NDT)
            g2_sb = load_pd("g2", g2, NDT)
            be2_sb = load_pd("be2", be2, NDT)

            def wtile(src_ap, name):
                t = wbig.tile([P, NDT, D], BF, tag="w", name=name)
                nc.sync.dma_start(t[:], src_ap)
                return t

            def layer_norm(ps_pool, src, dst, g_sb, be_sb, out_dma=None):
                """dst[:, do, :] = LN(src) over d_model (partition + do axes);
                per-token (free-axis) stats via ones-matmul column sums."""
                ps1 = ps_pool.tile([P, QCH], dt.float32, tag="ln", name="ps1")
                for do in range(NDT):
                    nc.tensor.matmul(ps1[:], onesr_sb[:], src[:, do, :],
                                     start=(do == 0), stop=(do == NDT - 1))
                ps2 = ps_pool.tile([P, QCH], dt.float32, tag="ln", name="ps2")
                for do in range(NDT):
                    sq = small.tile([P, QCH], BF, tag="sq")
                    nc.vector.tensor_tensor(sq[:], src[:, do, :], src[:, do, :],
                                            OP.mult)
                    nc.tensor.matmul(ps2[:], onesr_sb[:], sq[:],
                                     start=(do == 0), stop=(do == NDT - 1))
                mean = small.tile([P, QCH], BF, tag="mean")
                nc.vector.tensor_scalar(out=mean[:], in0=ps1[:], scalar1=1.0 / D,
                                        scalar2=None, op0=OP.mult)
                m2 = small.tile([P, QCH], BF, tag="m2")
                nc.vector.tensor_tensor(m2[:], mean[:], mean[:], OP.mult)
                var = small.tile([P, QCH], BF, tag="var")
                nc.vector.scalar_tensor_tensor(
                    out=var[:], in0=ps2[:], scalar=1.0 / D, in1=m2[:],
                    op0=OP.mult, op1=OP.subtract)
                sstd = small.tile([P, QCH], BF, tag="sstd")
                nc.scalar.activation(out=sstd[:], in_=var[:], func=AF.Sqrt,
                                     bias=eps_sb[:], scale=1.0)
                rstd = small.tile([P, QCH], BF, tag="rstd")
                with nc.allow_low_precision(reason="bf16 rstd, ~4e-3 rel ok"):
                    nc.vector.reciprocal(out=rstd[:], in_=sstd[:])
                for do in range(NDT):
                    t1 = small.tile([P, QCH], BF, tag="ln_t1")
                    nc.vector.tensor_tensor(t1[:], src[:, do, :], mean[:],
                                            OP.subtract)
                    nc.vector.tensor_tensor(t1[:], t1[:], rstd[:], OP.mult)
                    nc.vector.tensor_scalar(
                        out=dst[:, do, :], in0=t1[:],
                        scalar1=g_sb[:, do:do + 1], scalar2=be_sb[:, do:do + 1],
                        op0=OP.mult, op1=OP.add)
                    if out_dma is not None:
                        nc.sync.dma_start(out_dma[:, do, :], dst[:, do, :])

            ctxT = mid.tile([P, NDT, QCH], BF, tag="ctxT")
            yT = mid.tile([P, NDT, QCH], BF, tag="yT")
            hT = mid.tile([P, NDT, QCH], BF, tag="hT")

            with tc.tile_pool(name="attn", bufs=1) as attn:
                xT_sb = attn.tile([P, NDT, S], BF, tag="xT")
                nc.sync.dma_start(xT_sb[:, :, 0:QCH], xT[:, :, 0:QCH])
                kt_sb = attn.tile([P, NDT, S], BF, tag="kt")
                v_sb = attn.tile([P, KT, H, HD + 1], BF, tag="v")
                qT_sb = attn.tile([P, NDT, QCH], BF, tag="qT")
                nc.vector.memset(v_sb[:, :, :, HD], 1.0)

                # ---- projections ----
                with (
                    tc.tile_pool(name="psP", bufs=2, space="PSUM") as psP,
                ):
                    wq_t = wtile(wq[:], "wq_t")
                    nc.sync.dma_start(xT_sb[:, :, QCH:], xT[:, :, QCH:])
                    for do in range(NDT):
                        pq = psP.tile([P, QCH], dt.float32, tag="pq")
                        for k in range(NDT):
                            nc.tensor.matmul(
                                pq[:], wq_t[:, k, do * P:(do + 1) * P],
                                xT_sb[:, k, 0:QCH],
                                start=(k == 0), stop=(k == NDT - 1))
                        nc.vector.tensor_scalar(
                            out=qT_sb[:, do, :], in0=pq[:],
                            scalar1=bq_sb[:, do:do + 1], scalar2=None,
                            op0=OP.add)
                    wk_t = wtile(wk[:], "wk_t")
                    for do in range(NDT):
                        for np_ in range(2):
                            pk = psP.tile([P, 2 * QCH], dt.float32, tag="pk")
                            for half in range(2):
                                n = 2 * np_ + half
                                for k in range(NDT):
                                    nc.tensor.matmul(
                                        pk[:, half * QCH:(half + 1) * QCH],
                                        wk_t[:, k, do * P:(do + 1) * P],
                                        xT_sb[:, k, n * QCH:(n + 1) * QCH],
                                        start=(k == 0), stop=(k == NDT - 1))
                            nc.scalar.activation(
                                out=kt_sb[:, do, np_ * 2 * QCH:(np_ + 1) * 2 * QCH],
                                in_=pk[:], func=AF.Identity,
                                bias=bk_sb[:, do:do + 1], scale=1.0)
                    wv_t = wtile(wv[:], "wv_t")
                    # bulk constants — needed from attention onwards
                    tri_sb = consts.tile([P, 4, 2 * QCH], BF, tag="tri")
                    nc.sync.dma_start(tri_sb[:], tri[:])
                    onesr_sb = consts.tile([P, P], BF, tag="onesr")
                    nc.sync.dma_start(onesr_sb[:], onesr[:])
                    bvb_sb = consts.tile([P, D], BF, tag="bvb")
                    nc.sync.dma_start(bvb_sb[:], bvb[:])
                    for tt in range(KT):
                        pv = psP.tile([P, QCH], dt.float32, tag="pq",
                                      name="pv")
                        for k in range(NDT):
                            nc.tensor.matmul(
                                pv[:], xT_sb[:, k, tt * P:(tt + 1) * P],
                                wv_t[:, k, 0:QCH],
                                start=(k == 0), stop=(k == NDT - 1))
                        nc.vector.tensor_tensor(
                            v_sb[:, tt, 0:8, 0:HD],
                            pv[:].rearrange("p (h d) -> p h d", d=HD),
                            bvb_sb[:, 0:QCH].rearrange(
                                "p (h d) -> p h d", d=HD),
                            OP.add)

                # ---- attention: 8 head pairs, ctx accumulates in PSUM ----
                wo_t = wtile(wo[:], "wo_t")  # prefetch during attention
                with (
                    tc.tile_pool(name="psS", bufs=2, space="PSUM") as psS,
                    tc.tile_pool(name="psC", bufs=3, space="PSUM") as psC,
                ):
                    for a in range(H // 2):
                        pcs = [psC.tile([P, QCH], dt.float32, tag="pc",
                                        name=f"pc{i}") for i in range(2)]

                        def emit_scores(j, a=a):
                            psc = psS.tile([P, 2 * QCH], dt.float32,
                                           tag="psc", name="psc")
                            for i in range(2):
                                bp = i * HD
                                nc.tensor.matmul(
                                    psc[:, i * QCH:(i + 1) * QCH],
                                    kt_sb[bp:bp + HD, a, j * P:(j + 1) * P],
                                    qT_sb[bp:bp + HD, a, :],
                                    start=True, stop=True,
                                    tile_position=(bp, 0))
                            return psc

                        psc_cur = emit_scores(0)
                        for j in range(KT):
                            # software pipeline: next j's scores go to the PE
                            # ahead of this j's ctx so the PE never waits on
                            # the Scalar engine's exp
                            psc_next = emit_scores(j + 1) if j + 1 < KT else None
                            ex = expp.tile([P, 2 * QCH], BF, tag="exp")
                            nc.scalar.activation(
                                out=ex[:], in_=psc_cur[:], func=AF.Exp,
                                bias=kbias_sb[:, j:j + 1], scale=0.125)
                            if j < 4:
                                nc.vector.tensor_tensor(ex[:], ex[:],
                                                        tri_sb[:, j, :],
                                                        OP.mult)
                            for i in range(2):
                                nc.tensor.matmul(
                                    pcs[i][0:HD + 1, :], v_sb[:, j, 2 * a + i, :],
                                    ex[:, i * QCH:(i + 1) * QCH],
                                    start=(j == 0), stop=(j == KT - 1))
                            psc_cur = psc_next
                            # second half of the V projection (heads 8-15),
                            # interleaved into the PE stream of pairs 0-3
                            if a < 4 and j % 4 == 3:
                                tt = 4 * a + j // 4
                                pv1 = psC.tile([P, QCH], dt.float32, tag="pv",
                                               bufs=1, name="pv1")
                                for k in range(NDT):
                                    nc.tensor.matmul(
                                        pv1[:], xT_sb[:, k, tt * P:(tt + 1) * P],
                                        wv_t[:, k, QCH:2 * QCH],
                                        start=(k == 0), stop=(k == NDT - 1))
                                nc.vector.tensor_tensor(
                                    v_sb[:, tt, 8:16, 0:HD],
                                    pv1[:].rearrange("p (h d) -> p h d", d=HD),
                                    bvb_sb[:, QCH:2 * QCH].rearrange(
                                        "p (h d) -> p h d", d=HD),
                                    OP.add)
                        # copy raw ctx + recip out of PSUM fast (frees the
                        # pcs banks for the next pair), then normalize
                        # in-place off the PE's critical path
                        rcs = []
                        for i in range(2):
                            rc = small.tile([1, QCH], BF, tag=f"rc{i}",
                                            name="rc")
                            with nc.allow_low_precision(
                                    reason="bf16 recip colsum, ~4e-3 ok"):
                                nc.vector.reciprocal(out=rc[:],
                                                     in_=pcs[i][HD:HD + 1, :])
                            nc.vector.tensor_copy(
                                out=ctxT[i * HD:(i + 1) * HD, a, :],
                                in_=pcs[i][0:HD, :])
                            rcs.append(rc)
                        for i in range(2):
                            prcb = small.tile([P, QCH], BF, tag="prcb",
                                              name="prcb")
                            nc.gpsimd.partition_broadcast(prcb[:], rcs[i][:])
                            nc.vector.tensor_tensor(
                                ctxT[i * HD:(i + 1) * HD, a, :],
                                ctxT[i * HD:(i + 1) * HD, a, :],
                                prcb[i * HD:(i + 1) * HD, :], OP.mult)

                # ---- O proj + residual + LN1 ----
                w1q = [None] * 4
                w1q[0] = wtile(w1[:, :, 0:D], "w1q")  # prefetch
                with tc.tile_pool(name="psO", bufs=2, space="PSUM") as psO:
                    for do in range(NDT):
                        po = psO.tile([P, QCH], dt.float32, tag="po")
                        for k in range(NDT):
                            nc.tensor.matmul(
                                po[:], wo_t[:, k, do * P:(do + 1) * P],
                                ctxT[:, k, :],
                                start=(k == 0), stop=(k == NDT - 1))
                        nc.vector.scalar_tensor_tensor(
                            out=yT[:, do, :], in0=po[:],
                            scalar=bo_sb[:, do:do + 1],
                            in1=xT_sb[:, do, 0:QCH], op0=OP.add, op1=OP.add)
                    layer_norm(psO, yT, hT, g1_sb, be1_sb)

            # ---- FFN (intermediate stays in SBUF, bf16) ----
            with tc.tile_pool(name="ffnp", bufs=1) as ffnp:
                ff_sb = ffnp.tile([P, NFT, QCH], BF, tag="ff")
                y2T = ffnp.tile([P, NDT, QCH], BF, tag="y2T")
                outT = ffnp.tile([P, NDT, QCH], dt.float32, tag="outT")
                w2qs = {}
                with tc.tile_pool(name="psF1", bufs=4, space="PSUM") as psF1:
                    for ft in range(NFT):
                        if ft % 8 == 0 and ft // 8 < 3:
                            q = ft // 8 + 1
                            w1q[q] = wtile(w1[:, :, q * D:(q + 1) * D], "w1q")
                        if ft == 16:  # prefetch first W2 quarter
                            w2qs[0] = wbig.tile([P, NDT, D], BF, tag="w",
                                                name="w2q")
                            nc.sync.dma_start(w2qs[0][:], w2[:, 0:8, :])
                        pf = psF1.tile([P, QCH], dt.float32, tag="pf")
                        wt = w1q[ft // 8]
                        for k in range(NDT):
                            nc.tensor.matmul(
                                pf[:], wt[:, k, (ft % 8) * P:(ft % 8 + 1) * P],
                                hT[:, k, :],
                                start=(k == 0), stop=(k == NDT - 1))
                        nc.scalar.activation(
                            out=ff_sb[:, ft, :], in_=pf[:], func=AF.Relu,
                            bias=b1_sb[:, ft:ft + 1], scale=1.0)
                with tc.tile_pool(name="psF2", bufs=8, space="PSUM") as psF2:
                    accs = [psF2.tile([P, QCH], dt.float32, tag="acc",
                                      name=f"acc{do}") for do in range(NDT)]
                    for k in range(NFT):
                        if k % 8 == 0 and k > 0:
                            w2qs[k // 8] = wbig.tile([P, NDT, D], BF, tag="w",
                                                     name="w2q")
                            nc.sync.dma_start(w2qs[k // 8][:],
                                              w2[:, k:k + 8, :])
                        w2q = w2qs[k // 8]
                        for do in range(NDT):
                            nc.tensor.matmul(
                                accs[do][:], w2q[:, k % 8, do * P:(do + 1) * P],
                                ff_sb[:, k, :],
                                start=(k == 0), stop=(k == NFT - 1))
                    for do in range(NDT):
                        nc.vector.scalar_tensor_tensor(
                            out=y2T[:, do, :], in0=accs[do][:],
                            scalar=b2_sb[:, do:do + 1], in1=hT[:, do, :],
                            op0=OP.add, op1=OP.add)
                with tc.tile_pool(name="psL2", bufs=2, space="PSUM") as psL2:
                    layer_norm(psL2, y2T, outT, g2_sb, be2_sb, out_dma=out)

    nc.finalize()
    return nc


def _get_nc(debug=False):
    key = ("nc", debug)
    if key not in _CACHE:
        _CACHE[key] = _build(debug)
    return _CACHE[key]


def _prep_shared(Wq, bq, Wk, bk, Wv, bv, Wo, bo, W1, b1, W2, b2,
                 gamma1, beta1, gamma2, beta2):
    import ml_dtypes
    BF = ml_dtypes.bfloat16

    def wT(W):  # [D, N] -> [P, D//P, N] (d_in split over partitions)
        W = np.asarray(W, np.float32)
        kt = W.shape[0] // P
        return np.ascontiguousarray(
            W.reshape(kt, P, W.shape[1]).transpose(1, 0, 2).astype(BF))

    def pd(v):  # [N] -> [P, N//P] (per-partition layout, fp32)
        v = np.asarray(v, np.float32)
        return np.ascontiguousarray(v.reshape(-1, P).T)

    # triangular self-chunk masks, duplicated across the two 512 halves
    p_ = np.arange(P)[:, None]
    f_ = np.arange(QCH)[None, :]
    tri = np.zeros((P, 4, 2 * QCH), np.float32)
    for j in range(4):
        m = ((P * j + p_) <= f_).astype(np.float32)
        tri[:, j, 0:QCH] = m
        tri[:, j, QCH:] = m
    return {
        "wq": wT(Wq), "wk": wT(Wk), "wv": wT(Wv), "wo": wT(Wo),
        "w1": wT(W1), "w2": wT(W2),
        "bq": pd(bq), "bk": pd(bk), "bo": pd(bo),
        "b1": pd(b1), "b2": pd(b2),
        "g1": pd(gamma1), "be1": pd(beta1), "g2": pd(gamma2), "be2": pd(beta2),
        "bvb": np.ascontiguousarray(
            np.broadcast_to(np.asarray(bv, np.float32), (P, D)).astype(BF)),
        "tri": tri.astype(BF),
        "onesr": np.ones((P, P), BF),
    }


def kernel(x, mask, Wq, bq, Wk, bk, Wv, bv, Wo, bo, W1, b1, W2, b2,
           gamma1, beta1, gamma2, beta2, _trace=False, _debug=False,
           _mm_dtype=None):
    import ml_dtypes
    from concourse.bass_utils import run_bass_kernel_spmd

    BF = ml_dtypes.bfloat16
    nc = _get_nc(_debug)
    x = np.ascontiguousarray(np.asarray(x, dtype=np.float32))
    shared = _prep_shared(Wq, bq, Wk, bk, Wv, bv, Wo, bo, W1, b1, W2, b2,
                          gamma1, beta1, gamma2, beta2)
    in_maps = []
    for c in range(NCORES):
        b, i = divmod(c, NCORES // B)
        q0 = i * QCH
        xb_rot = np.concatenate(
            [x[b, q0:q0 + QCH], x[b, :q0], x[b, q0 + QCH:]], axis=0)
        # [S, D] -> [P, NDT, S] transposed layout, bf16
        xTc = xb_rot.T.reshape(NDT, P, S).transpose(1, 0, 2).astype(BF)
        kb = np.zeros((P, KT), np.float32)
        n_ok = 4 + q0 // P  # self tiles + past tiles
        kb[:, n_ok:] = NEG
        in_maps.append({
            "xT": np.ascontiguousarray(xTc),
            "kbias": kb,
            **shared,
        })
    res = run_bass_kernel_spmd(nc, in_maps, core_ids=list(range(NCORES)),
                               trace=_trace)
    out = np.empty((B, S, D), np.float32)
    for c in range(NCORES):
        b, i = divmod(c, NCORES // B)
        r = np.asarray(res.results[c]["out"], np.float32)  # [P, NDT, QCH]
        out[b, i * QCH:(i + 1) * QCH] = (
            r.transpose(2, 1, 0).reshape(QCH, D))
    if _trace:
        _CACHE["last_result"] = res
    return out
